# revision 19
# baseline (speedup 1.0000x reference)
"""Trainium2 Bass kernel for masked multi-modal causal dot-product attention.

Computation (reference):
  Q = mlp(x1, Wq)               # (4096, 64), 3 linear layers, relu between
  for m in 0..3:
    K_m = mlp(x_m, Wk[m])       # (4096, 64)
    mask_m[i,j] = t2_m[j] <= t1[i]   (timestamps sorted -> staircase mask)
    acc += ((Q @ K_m.T) * mask_m) @ x_m[:, :2]
  out = acc  # (1, 4096, 2)

Sharding: 8 cores = 4 modalities x 2 query-parity halves (queries interleaved
by 128-chunks for load balance). One SPMD program; per-core variation lives in
the input tensors.

Algorithm (prefix-sum restructure): since both t1 and t2 are sorted, the mask
is a monotone staircase. For each 128-query chunk, key tiles split into
fully-visible / boundary / invisible. The fully-visible mass uses
associativity:  sum_vis (Q.K_j) V_j = Q @ P  with  P = sum_j K_j^T V_j
a prefix sum over 128-key tiles of tiny [64,2] matrices. Only the ~2-4
boundary tiles per query chunk need explicit masked S tiles ([128,128]).

Device pipeline per core:
  - PE warm-up matmuls while DMAs stream (HAM un-throttle).
  - MLPs on stacked halves with block-diagonal weights (K=128 packing),
    f32r; final layers emit K^T pair tiles (kTblk) and Q^T (qT2) in bf16,
    plus an fp32 Q^T copy (qTf) for the base matmuls.
  - Delta pass: K-natural tiles reconstructed from the packed hidden state
    via two placed matmuls per pair tile (even/odd keys on partition
    halves), then delta_t = Knat_t^T @ V_t accumulated into one PSUM bank.
  - Prefix chain on DVE -> PAll[c] = sum_{t<c} delta_t  (fp32).
  - Per 512-query block: PSUM bank memset; 4 base matmuls (PAll[F]^T @ Q^T,
    f32r); boundary units: S tile (bf16) -> fused (t1>=t2)*S on DVE ->
    AV matmul; all accumulate into the same bank; copy out.

Final-layer K bias is folded exactly: boundary tiles use biased K (kTblk);
the base region correction (Q.b2)*prefix(sum V) is identically zero here
(reference biases are zero) but is applied on host if ever nonzero.
"""

import os
import sys
from collections import deque

import ml_dtypes
import numpy as np

sys.path.insert(0, "/opt/trn_rl_repo")

T = 4096
D = 64
M = 4
NLIN = 3
NQ = 2048          # packed queries per core
CHUNK = 128        # keys per pair tile (64 even + 64 odd)
NPAIR = T // CHUNK  # 32 pair tiles
IBLK = 512         # query block for MLPs / out banks
NBLK = NQ // IBLK  # 4 query blocks per core
QC = 128           # boundary query-chunk granularity
NQC = NQ // QC     # 16 query chunks per core

LAST_RESULTS = None


def _build_program(F128, J128):
    """F128[qc]: pair tiles < F128 are fully visible for every core's chunk
    qc; F128 <= jt < J128[qc] get the on-device mask (universal bounds)."""
    import concourse.bacc as bacc
    import concourse.mybir as mybir
    import concourse.tile as tile

    f32 = mybir.dt.float32
    f32r = mybir.dt.float32r
    bf16 = mybir.dt.bfloat16
    Relu = mybir.ActivationFunctionType.Relu
    Identity = mybir.ActivationFunctionType.Identity
    is_ge = mybir.AluOpType.is_ge
    add = mybir.AluOpType.add
    amax = mybir.AluOpType.max
    mult = mybir.AluOpType.mult

    nc = bacc.Bacc("TRN2", target_bir_lowering=False, debug=False, num_devices=8)

    xqT = nc.dram_tensor("xqT", [128, NQ // 2], bf16, kind="ExternalInput")
    xkT = nc.dram_tensor("xkT", [128, T // 2], bf16, kind="ExternalInput")
    xkv = nc.dram_tensor("xkv", [128, NPAIR * 2], bf16, kind="ExternalInput")
    xkvT = nc.dram_tensor("xkvT", [128, 16 * 4], bf16, kind="ExternalInput")
    xt2 = nc.dram_tensor("xt2", [128, NPAIR], f32, kind="ExternalInput")
    t1p = nc.dram_tensor("t1p", [1, NQ], f32, kind="ExternalInput")
    wq = nc.dram_tensor("wq", [128, 4 * 128], bf16, kind="ExternalInput")
    bq = nc.dram_tensor("bq", [128, 4], f32, kind="ExternalInput")
    wk = nc.dram_tensor("wk", [128, NLIN * 128], bf16, kind="ExternalInput")
    bk = nc.dram_tensor("bk", [128, NLIN], f32, kind="ExternalInput")
    out = nc.dram_tensor("out", [2, NQ], f32, kind="ExternalOutput")

    def rr(ap):
        return ap.bitcast(f32r)

    with tile.TileContext(nc) as tc:
        with (
            tc.tile_pool(name="const", bufs=1) as const,
            tc.tile_pool(name="hq", bufs=2) as hqp,
            tc.tile_pool(name="hk", bufs=2) as hkp,
            tc.tile_pool(name="knp", bufs=16) as knp,
            tc.tile_pool(name="spool", bufs=4) as spool,
            tc.tile_pool(name="mkp", bufs=4) as mkp,
            tc.tile_pool(name="ps_mlp", bufs=2, space="PSUM") as ps_mlp,
            tc.tile_pool(name="ps_d", bufs=1, space="PSUM") as ps_d,
            tc.tile_pool(name="ps_s", bufs=3, space="PSUM") as ps_s,
            tc.tile_pool(name="ps_o", bufs=2, space="PSUM") as ps_o,
        ):
            # ---- PE warm-up: dummy bf16 matmuls so HAM un-throttles the PE
            # clock (4/8 -> 8/8) while the input DMAs stream in.
            wu = const.tile([128, 512], bf16)
            nc.gpsimd.memset(wu[:], 1.0)
            for i in range(10):
                wps = ps_mlp.tile([128, 512], f32, tag="ps")
                nc.tensor.matmul(
                    wps[:], wu[:, 0:128], wu[:], start=True, stop=True,
                    skip_group_check=True,
                )

            # ---- inputs -> SBUF (ordered so the MLPs can start ASAP)
            wq_sb = const.tile([128, 4, 128], bf16)
            nc.sync.dma_start(wq_sb[:], wq[:].rearrange("p (l e) -> p l e", l=4))
            bq_sb = const.tile([128, 4], f32)
            nc.sync.dma_start(bq_sb[:], bq[:])
            wk_sb = const.tile([128, NLIN, 128], bf16)
            nc.sync.dma_start(wk_sb[:], wk[:].rearrange("p (l e) -> p l e", l=NLIN))
            bk_sb = const.tile([128, NLIN], f32)
            nc.sync.dma_start(bk_sb[:], bk[:])

            xqT_sb = const.tile([128, NQ // 2], bf16)
            xkT_sb = const.tile([128, T // 2], bf16)
            order = [("k", 0), ("k", 1), ("q", 0), ("k", 2), ("k", 3), ("q", 1)]
            for which, nb in order:
                sl = slice(nb * IBLK, (nb + 1) * IBLK)
                if which == "k":
                    nc.sync.dma_start(xkT_sb[:, sl], xkT[:, sl])
                else:
                    nc.sync.dma_start(xqT_sb[:, sl], xqT[:, sl])

            xkv_sb = const.tile([128, NPAIR, 2], bf16)
            nc.sync.dma_start(xkv_sb[:], xkv[:].rearrange("p (c f) -> p c f", f=2))
            xkvT_sb = const.tile([128, 16, 4], bf16)
            nc.sync.dma_start(xkvT_sb[:], xkvT[:].rearrange("p (c f) -> p c f", f=4))
            xt2_sb = const.tile([128, NPAIR], f32)
            nc.sync.dma_start(xt2_sb[:], xt2[:])
            t1b_sb = const.tile([CHUNK, NQ], f32)
            for nb in range(NBLK):
                sl = slice(nb * IBLK, (nb + 1) * IBLK)
                nc.sync.dma_start(t1b_sb[:, sl], t1p[:, sl].partition_broadcast(CHUNK))

            out_sb = const.tile([2, NQ], f32)

            # ---- blocked K^T target: pair tiles with block-diagonal layout
            kTblk = const.tile([128, NPAIR, CHUNK], bf16)
            zeros_sb = const.tile([128, NPAIR, 64], bf16)
            nc.vector.memset(zeros_sb[:], 0.0)
            nc.vector.tensor_copy(kTblk[0:64, :, 64:128], zeros_sb[0:64])
            nc.scalar.copy(kTblk[64:128, :, 0:64], zeros_sb[64:128])
            qT2 = const.tile([128, NQ], bf16)
            qTf = const.tile([64, NQ], f32r)
            PAll = const.tile([64, NPAIR + 1, 2], f32r)

            nc.vector.tensor_copy(PAll[:, 0, :], zeros_sb[0:64, 0, 0:2])

            # ---- stacked MLPs (block-diagonal weights, both halves at once)
            def epilogue(dst, ps, bias, layer, eng):
                if eng == "act":
                    func = Relu if layer < NLIN - 1 else Identity
                    nc.scalar.activation(dst, ps, func, bias=bias)
                elif layer < NLIN - 1:
                    nc.vector.tensor_scalar(dst, ps, bias, 0.0, op0=add, op1=amax)
                else:
                    nc.vector.tensor_scalar(dst, ps, bias, None, op0=add)

            def mlp_hidden(cur, w_sb, b_sb, pool, nt, layer, eng):
                nxt = pool.tile([128, nt], bf16, tag="h")
                for nb in range(nt // IBLK):
                    sl = slice(nb * IBLK, (nb + 1) * IBLK)
                    ps = ps_mlp.tile([128, IBLK], f32, tag="ps")
                    nc.tensor.matmul(
                        ps[:], w_sb[:, layer, :], cur[:, sl], start=True, stop=True
                    )
                    epilogue(nxt[:, sl], ps[:], b_sb[:, layer : layer + 1], layer, eng)
                return nxt

            hk, hq = xkT_sb, xqT_sb
            for layer in range(NLIN - 1):
                hk = mlp_hidden(hk, wk_sb, bk_sb, hkp, T // 2, layer, "act")
                hq = mlp_hidden(hq, wq_sb, bq_sb, hqp, NQ // 2, layer, "dve")

            # final K layer: write straight into block-diagonal pair tiles
            eng_flip = 0
            for nb in range(T // 2 // IBLK):
                sl = slice(nb * IBLK, (nb + 1) * IBLK)
                ps = ps_mlp.tile([128, IBLK], f32, tag="ps")
                nc.tensor.matmul(
                    ps[:], wk_sb[:, NLIN - 1, :], hk[:, sl], start=True, stop=True
                )
                psv = ps[:].rearrange("p (a e) -> p a e", e=64)
                pair = slice(8 * nb, 8 * nb + 8)
                bias = bk_sb[:, NLIN - 1 : NLIN]
                for half, csl in ((slice(0, 64), slice(0, 64)),
                                  (slice(64, 128), slice(64, 128))):
                    dst = kTblk[half, pair, csl]
                    src = psv[half, :, :]
                    if eng_flip % 2 == 0:
                        nc.scalar.activation(dst, src, Identity, bias=bias[half])
                    else:
                        nc.vector.tensor_scalar(dst, src, bias[half], None, op0=add)
                    eng_flip += 1

            # final Q layer: replicate Q^T onto both partition halves (bf16)
            # plus an fp32 copy of the top half for the base matmuls
            for nb in range(NQ // 2 // IBLK):
                sl = slice(nb * IBLK, (nb + 1) * IBLK)
                bias = bq_sb[:, NLIN - 1 : NLIN]
                for rep in range(2):
                    ps = ps_mlp.tile([128, IBLK], f32, tag="ps")
                    nc.tensor.matmul(
                        ps[:], wq_sb[:, 2 + rep, :], hq[:, sl], start=True, stop=True
                    )
                    osl = slice(rep * (NQ // 2) + nb * IBLK,
                                rep * (NQ // 2) + (nb + 1) * IBLK)
                    epilogue(qT2[:, osl], ps[:], bias, NLIN - 1,
                             "act" if rep else "dve")
                    epilogue(qTf[:, osl], ps[0:64, :], bias[0:64], NLIN - 1,
                             "dve" if rep else "act")

            # ---- delta pass: Knat tiles from packed hidden state, then
            # delta_t = Knat_t^T @ V_t into one PSUM bank [64, NPAIR*2]
            dps = ps_d.tile([64, NPAIR * 2], f32)
            kns = []
            for cc in range(16):
                psKt = ps_mlp.tile([128, 128], f32, tag="ps")
                hs = hk[:, 128 * cc : 128 * cc + 128]
                nc.tensor.matmul(
                    psKt[:], hs, wk_sb[:, 2, :],
                    start=True, stop=True, skip_group_check=True,
                )
                kn = knp.tile([128, 128], bf16, tag="kn")
                if cc % 2 == 0:
                    nc.scalar.copy(kn[:], psKt[:])
                else:
                    nc.vector.tensor_copy(kn[:], psKt[:])
                kns.append(kn)
            # all delta matmuls back-to-back into one PSUM bank
            for cc in range(16):
                kn = kns[cc]
                for h, (hp, vc) in enumerate(
                    ((slice(0, 64), slice(0, 2)), (slice(0, 64), slice(2, 4)),
                     (slice(64, 128), slice(0, 2)), (slice(64, 128), slice(2, 4)))
                ):
                    t = 2 * cc + (h // 2)
                    fc = slice(0, 64) if h % 2 == 0 else slice(64, 128)
                    nc.tensor.matmul(
                        dps[:, 2 * t : 2 * t + 2], kn[hp, fc], xkvT_sb[hp, cc, vc],
                        start=(cc == 0 and h == 0),
                        stop=(cc == 15 and h == 3), skip_group_check=True,
                    )

            # ---- prefix chain on DVE: PAll[c] = PAll[c-1] + delta_{c-1}
            for c in range(1, NPAIR + 1):
                nc.vector.tensor_add(
                    PAll[:, c, :], PAll[:, c - 1, :],
                    dps[:, 2 * (c - 1) : 2 * c],
                )

            # ---- main loop: per 512-block bank; base matmuls + boundary units
            for b in range(NBLK):
                bsl = slice(b * IBLK, (b + 1) * IBLK)
                ov = ps_o.tile([2, IBLK], f32)
                nc.vector.memset(ov[:], 0.0)
                units = []
                for q in range(4):
                    qc = 4 * b + q
                    for jt in range(F128[qc], J128[qc]):
                        units.append((q, qc, jt))
                # base matmuls (start=False over the memset bank)
                for q in range(4):
                    qc = 4 * b + q
                    osl = slice(q * QC, (q + 1) * QC)
                    nc.tensor.matmul(
                        ov[:, osl], PAll[:, F128[qc], :],
                        qTf[:, qc * QC : (qc + 1) * QC],
                        start=False, stop=False, skip_group_check=True,
                    )
                # boundary units: S-matmuls and AVs in runs of 3 so each
                # kind streams back-to-back on the PE
                pend = deque()

                def emit_av(q, jt, s_sb, last):
                    osl = slice(q * QC, (q + 1) * QC)
                    nc.tensor.matmul(
                        ov[:, osl], xkv_sb[:, jt, :], s_sb[:],
                        start=False, stop=last, skip_group_check=True,
                    )

                for i, (q, qc, jt) in enumerate(units):
                    qsl = slice(qc * QC, (qc + 1) * QC)
                    sp = ps_s.tile([CHUNK, QC], f32)
                    nc.tensor.matmul(
                        sp[:], kTblk[:, jt, :], qT2[:, qsl],
                        start=True, stop=True, skip_group_check=True,
                    )
                    mk = mkp.tile([CHUNK, QC], f32, tag="mk")
                    nc.gpsimd.tensor_scalar(
                        mk[:], t1b_sb[:, qsl], xt2_sb[:, jt : jt + 1], None,
                        op0=is_ge,
                    )
                    s_sb = spool.tile([CHUNK, QC], bf16)
                    nc.vector.tensor_mul(s_sb[:], sp[:], mk[:])
                    pend.append((q, jt, s_sb, i == len(units) - 1))
                    if i % 3 == 2:
                        while len(pend) > 3:
                            emit_av(*pend.popleft())
                while pend:
                    emit_av(*pend.popleft())
                nc.scalar.copy(out_sb[:, bsl], ov[:])

            nc.sync.dma_start(out[:], out_sb[:])

    nc.compile()
    return nc


def _stack_keys(a):
    """[T, ...] -> even/odd 64-chunk split stacked on a new leading axis."""
    v = a.reshape(NPAIR, 2, 64, *a.shape[1:])
    return v[:, 0], v[:, 1]  # each [NPAIR, 64, ...]


def kernel(x1, x2, x3, x4, Wq_w, Wq_b, Wk_w, Wk_b):
    from concourse.bass_utils import run_bass_kernel_spmd

    global LAST_RESULTS

    xs = [np.asarray(a, dtype=np.float32)[0, 0] for a in (x1, x2, x3, x4)]
    Wq_w = np.asarray(Wq_w, dtype=np.float32)
    Wq_b = np.asarray(Wq_b, dtype=np.float32)
    Wk_w = np.asarray(Wk_w, dtype=np.float32)
    Wk_b = np.asarray(Wk_b, dtype=np.float32)

    t1 = xs[0][:, -1]
    t2s = [x[:, -1] for x in xs]

    # ---- universal chunk classification (exact, quantified over all cores)
    F128 = [NPAIR] * NQC
    J128 = [0] * NQC
    for qc in range(NQC):
        for p in range(2):
            oc = 2 * qc + p
            lo, hi = t1[128 * oc], t1[128 * oc + 127]
            for m in range(M):
                nfull = int(np.searchsorted(t2s[m], lo, side="right"))
                nvis = int(np.searchsorted(t2s[m], hi, side="right"))
                F128[qc] = min(F128[qc], nfull // CHUNK)
                J128[qc] = max(J128[qc], -(-nvis // CHUNK))
        J128[qc] = max(J128[qc], F128[qc])

    nc = _build_program(F128, J128)

    # ---- host packing
    perm = np.empty((2, NQ), dtype=np.int64)
    for p in range(2):
        perm[p] = np.concatenate(
            [np.arange(128 * (2 * k + p), 128 * (2 * k + p) + 128) for k in range(16)]
        )

    def blockdiag(Wl):
        b = np.zeros((128, 128), np.float32)
        b[:64, :64] = Wl
        b[64:, 64:] = Wl
        return b

    # Q weights: layers 0,1 blockdiag; final as [[W,W],[0,0]] and [[0,0],[W,W]]
    wq_h = np.zeros((4, 128, 128), np.float32)
    for l in range(NLIN - 1):
        wq_h[l] = blockdiag(Wq_w[l])
    wq_h[2, :64, :64] = Wq_w[2]
    wq_h[2, :64, 64:] = Wq_w[2]
    wq_h[3, 64:, :64] = Wq_w[2]
    wq_h[3, 64:, 64:] = Wq_w[2]
    wq_h = np.ascontiguousarray(wq_h.transpose(1, 0, 2).reshape(128, 4 * 128))
    bq_h = np.tile(Wq_b.T, (2, 1))  # [128, 3]
    bq_h = np.ascontiguousarray(
        np.concatenate([bq_h, bq_h[:, 2:3]], axis=1)
    )  # [128, 4]

    x1T = np.ascontiguousarray(xs[0].T)

    in_maps = []
    for c in range(8):
        m, p = c // 2, c % 2
        xm = xs[m]
        # key-side stacking: even/odd 64-chunks
        ev, od = _stack_keys(xm)  # [NPAIR, 64, D] each
        xkT_h = np.concatenate(
            [
                ev.reshape(T // 2, D).T,   # [64, 2048]
                od.reshape(T // 2, D).T,
            ],
            axis=0,
        )  # [128, 2048]
        xkv_h = np.concatenate(
            [ev[:, :, 0:2], od[:, :, 0:2]], axis=1
        )  # [NPAIR, 128, 2]
        xkv_h = np.ascontiguousarray(
            xkv_h.transpose(1, 0, 2).reshape(128, NPAIR * 2)
        ).astype(ml_dtypes.bfloat16)
        v4 = xm[:, 0:2].reshape(16, 4, 64, 2)   # [chunk256, B, 64, 2]
        xkvT_h = np.concatenate(
            [np.concatenate([v4[:, 0], v4[:, 2]], axis=1)[:, :, None, :],
             np.concatenate([v4[:, 1], v4[:, 3]], axis=1)[:, :, None, :]],
            axis=2,
        )  # [16, 128, 2pairs, 2]
        xkvT_h = np.ascontiguousarray(
            xkvT_h.transpose(1, 0, 2, 3).reshape(128, 16 * 4)
        ).astype(ml_dtypes.bfloat16)
        xt2_h = np.concatenate(
            [ev[:, :, D - 1], od[:, :, D - 1]], axis=1
        ).T  # [128, NPAIR]

        wk_h = np.stack([blockdiag(Wk_w[m][l]) for l in range(NLIN)])
        wk_h = np.ascontiguousarray(wk_h.transpose(1, 0, 2).reshape(128, NLIN * 128))
        bk_h = np.ascontiguousarray(np.tile(Wk_b[m].T, (2, 1)))  # [128, 3]

        # query-side: parity packing then [first half | second half] stacking
        xq = x1T[:, perm[p]]  # [64, 2048]
        xqT_h = np.concatenate([xq[:, : NQ // 2], xq[:, NQ // 2 :]], axis=0)

        in_maps.append(
            {
                "xqT": np.ascontiguousarray(xqT_h).astype(ml_dtypes.bfloat16),
                "xkT": np.ascontiguousarray(xkT_h).astype(ml_dtypes.bfloat16),
                "xkv": xkv_h,
                "xkvT": xkvT_h,
                "xt2": np.ascontiguousarray(xt2_h),
                "t1p": np.ascontiguousarray(t1[perm[p]][None, :]),
                "wq": wq_h.astype(ml_dtypes.bfloat16),
                "bq": bq_h,
                "wk": wk_h.astype(ml_dtypes.bfloat16),
                "bk": bk_h,
            }
        )

    res = run_bass_kernel_spmd(nc, in_maps, core_ids=list(range(8)))
    LAST_RESULTS = res

    # ---- gather: sum over modalities, unpermute parity chunks, transpose
    acc = np.zeros((2, T), dtype=np.float32)
    for c in range(8):
        m, p = c // 2, c % 2
        acc[:, perm[p]] += res.results[c]["out"]
    out = np.ascontiguousarray(acc.T)[None]

    # ---- exact host correction for a nonzero final K bias in the base
    # region (the device prefix is bias-free; zero for reference inputs)
    if np.any(Wk_b[:, NLIN - 1]):
        def mlp(x, Ws, bs):
            h = x
            for i in range(Ws.shape[0]):
                h = h @ Ws[i] + bs[i]
                if i < Ws.shape[0] - 1:
                    h = np.maximum(h, 0.0)
            return h

        Q = mlp(xs[0], Wq_w, Wq_b)
        for m in range(M):
            qb = Q @ Wk_b[m][NLIN - 1]          # (T,)
            csV = np.concatenate(
                [np.zeros((1, 2), np.float32),
                 np.cumsum(xs[m][:, :2].reshape(NPAIR, CHUNK, 2).sum(1), axis=0)]
            )  # (NPAIR+1, 2)
            for p in range(2):
                for qc in range(NQC):
                    idx = perm[p][qc * QC : (qc + 1) * QC]
                    out[0, idx] += qb[idx, None] * csV[F128[qc]][None, :]
    return out


# revision 20
# speedup vs baseline: 2.1318x; 2.1318x over previous
"""Trainium2 Bass kernel for masked multi-modal causal dot-product attention.

Computation (reference):
  Q = mlp(x1, Wq)               # (4096, 64), 3 linear layers, relu between
  for m in 0..3:
    K_m = mlp(x_m, Wk[m])       # (4096, 64)
    mask_m[i,j] = t2_m[j] <= t1[i]   (timestamps sorted -> staircase mask)
    acc += ((Q @ K_m.T) * mask_m) @ x_m[:, :2]
  out = acc  # (1, 4096, 2)

Sharding: 8 cores = 4 modalities x 2 query-parity halves (queries interleaved
by 128-chunks for load balance). One SPMD program; per-core variation lives in
the input tensors.

Algorithm (prefix-sum restructure): since both t1 and t2 are sorted, the mask
is a monotone staircase. For each 128-query chunk, key tiles split into
fully-visible / boundary / invisible. The fully-visible mass uses
associativity:  sum_vis (Q.K_j) V_j = Q @ P  with  P = sum_j K_j^T V_j
a prefix sum over 128-key tiles of tiny [64,2] matrices. Only the ~2-4
boundary tiles per query chunk need explicit masked S tiles ([128,128]).

Device pipeline per core:
  - PE warm-up matmuls while DMAs stream (HAM un-throttle).
  - MLPs on stacked halves with block-diagonal weights (K=128 packing),
    f32r; final layers emit K^T pair tiles (kTblk) and Q^T (qT2) in bf16,
    plus an fp32 Q^T copy (qTf) for the base matmuls.
  - Delta pass: K-natural tiles reconstructed from the packed hidden state
    via two placed matmuls per pair tile (even/odd keys on partition
    halves), then delta_t = Knat_t^T @ V_t accumulated into one PSUM bank.
  - Prefix chain on DVE -> PAll[c] = sum_{t<c} delta_t  (fp32).
  - Per 512-query block: PSUM bank memset; 4 base matmuls (PAll[F]^T @ Q^T,
    f32r); boundary units: S tile (bf16) -> fused (t1>=t2)*S on DVE ->
    AV matmul; all accumulate into the same bank; copy out.

Final-layer K bias is folded exactly: boundary tiles use biased K (kTblk);
the base region correction (Q.b2)*prefix(sum V) is identically zero here
(reference biases are zero) but is applied on host if ever nonzero.
"""

import os
import sys
from collections import deque

import ml_dtypes
import numpy as np

sys.path.insert(0, "/opt/trn_rl_repo")

T = 4096
D = 64
M = 4
NLIN = 3
NQ = 2048          # packed queries per core
CHUNK = 128        # keys per pair tile (64 even + 64 odd)
NPAIR = T // CHUNK  # 32 pair tiles
IBLK = 512         # query block for MLPs / out banks
NBLK = NQ // IBLK  # 4 query blocks per core
QC = 128           # boundary query-chunk granularity
NQC = NQ // QC     # 16 query chunks per core

LAST_RESULTS = None


def _build_program(F128, J128):
    """F128[qc]: pair tiles < F128 are fully visible for every core's chunk
    qc; F128 <= jt < J128[qc] get the on-device mask (universal bounds)."""
    import concourse.bacc as bacc
    import concourse.mybir as mybir
    import concourse.tile as tile

    f32 = mybir.dt.float32
    f32r = mybir.dt.float32r
    bf16 = mybir.dt.bfloat16
    Relu = mybir.ActivationFunctionType.Relu
    Identity = mybir.ActivationFunctionType.Identity
    is_ge = mybir.AluOpType.is_ge
    add = mybir.AluOpType.add
    amax = mybir.AluOpType.max
    mult = mybir.AluOpType.mult

    nc = bacc.Bacc("TRN2", target_bir_lowering=False, debug=False, num_devices=8)

    xqT = nc.dram_tensor("xqT", [128, NQ // 2], bf16, kind="ExternalInput")
    xkT = nc.dram_tensor("xkT", [128, T // 2], bf16, kind="ExternalInput")
    xkv = nc.dram_tensor("xkv", [128, NPAIR * 2], bf16, kind="ExternalInput")
    xkvT = nc.dram_tensor("xkvT", [128, 16 * 4], bf16, kind="ExternalInput")
    xt2 = nc.dram_tensor("xt2", [128, NPAIR], f32, kind="ExternalInput")
    t1p = nc.dram_tensor("t1p", [1, NQ], f32, kind="ExternalInput")
    wq = nc.dram_tensor("wq", [128, 4 * 128], bf16, kind="ExternalInput")
    bq = nc.dram_tensor("bq", [128, 4], f32, kind="ExternalInput")
    wk = nc.dram_tensor("wk", [128, NLIN * 128], bf16, kind="ExternalInput")
    bk = nc.dram_tensor("bk", [128, NLIN], f32, kind="ExternalInput")
    out = nc.dram_tensor("out", [2, NQ], f32, kind="ExternalOutput")

    def rr(ap):
        return ap.bitcast(f32r)

    with tile.TileContext(nc) as tc:
        with (
            tc.tile_pool(name="const", bufs=1) as const,
            tc.tile_pool(name="hq", bufs=2) as hqp,
            tc.tile_pool(name="hk", bufs=2) as hkp,
            tc.tile_pool(name="knp", bufs=16) as knp,
            tc.tile_pool(name="spool", bufs=4) as spool,
            tc.tile_pool(name="mkp", bufs=4) as mkp,
            tc.tile_pool(name="ps_mlp", bufs=2, space="PSUM") as ps_mlp,
            tc.tile_pool(name="ps_d", bufs=1, space="PSUM") as ps_d,
            tc.tile_pool(name="ps_s", bufs=3, space="PSUM") as ps_s,
            tc.tile_pool(name="ps_o", bufs=2, space="PSUM") as ps_o,
        ):
            # ---- PE warm-up: dummy bf16 matmuls so HAM un-throttles the PE
            # clock (4/8 -> 8/8) while the input DMAs stream in.
            wu = const.tile([128, 512], bf16)
            nc.gpsimd.memset(wu[:], 1.0)
            for i in range(10):
                wps = ps_mlp.tile([128, 512], f32, tag="ps")
                nc.tensor.matmul(
                    wps[:], wu[:, 0:128], wu[:], start=True, stop=True,
                    skip_group_check=True,
                )

            # ---- inputs -> SBUF (ordered so the MLPs can start ASAP)
            wq_sb = const.tile([128, 4, 128], bf16)
            nc.sync.dma_start(wq_sb[:], wq[:].rearrange("p (l e) -> p l e", l=4))
            bq_sb = const.tile([128, 4], f32)
            nc.sync.dma_start(bq_sb[:], bq[:])
            wk_sb = const.tile([128, NLIN, 128], bf16)
            nc.sync.dma_start(wk_sb[:], wk[:].rearrange("p (l e) -> p l e", l=NLIN))
            bk_sb = const.tile([128, NLIN], f32)
            nc.sync.dma_start(bk_sb[:], bk[:])

            xqT_sb = const.tile([128, NQ // 2], bf16)
            xkT_sb = const.tile([128, T // 2], bf16)
            order = [("k", 0), ("k", 1), ("q", 0), ("k", 2), ("k", 3), ("q", 1)]
            for which, nb in order:
                sl = slice(nb * IBLK, (nb + 1) * IBLK)
                if which == "k":
                    nc.sync.dma_start(xkT_sb[:, sl], xkT[:, sl])
                else:
                    nc.sync.dma_start(xqT_sb[:, sl], xqT[:, sl])

            xkv_sb = const.tile([128, NPAIR, 2], bf16)
            nc.sync.dma_start(xkv_sb[:], xkv[:].rearrange("p (c f) -> p c f", f=2))
            xkvT_sb = const.tile([128, 16, 4], bf16)
            nc.sync.dma_start(xkvT_sb[:], xkvT[:].rearrange("p (c f) -> p c f", f=4))
            xt2_sb = const.tile([128, NPAIR], f32)
            nc.sync.dma_start(xt2_sb[:], xt2[:])
            t1b_sb = const.tile([CHUNK, NQ], f32)
            for nb in range(NBLK):
                sl = slice(nb * IBLK, (nb + 1) * IBLK)
                nc.sync.dma_start(t1b_sb[:, sl], t1p[:, sl].partition_broadcast(CHUNK))

            out_sb = const.tile([2, NQ], f32)

            # ---- blocked K^T target: pair tiles with block-diagonal layout
            kTblk = const.tile([128, NPAIR, CHUNK], bf16)
            zeros_sb = const.tile([128, NPAIR, 64], bf16)
            nc.vector.memset(zeros_sb[:], 0.0)
            nc.vector.tensor_copy(kTblk[0:64, :, 64:128], zeros_sb[0:64])
            nc.scalar.copy(kTblk[64:128, :, 0:64], zeros_sb[64:128])
            qT2 = const.tile([128, NQ], bf16)
            qTf = const.tile([64, NQ], f32r)
            PAll = const.tile([64, NPAIR + 1, 2], f32r)

            nc.vector.tensor_copy(PAll[:, 0, :], zeros_sb[0:64, 0, 0:2])

            # ---- stacked MLPs (block-diagonal weights, both halves at once)
            def epilogue(dst, ps, bias, layer, eng):
                if eng == "act":
                    func = Relu if layer < NLIN - 1 else Identity
                    nc.scalar.activation(dst, ps, func, bias=bias)
                elif layer < NLIN - 1:
                    nc.vector.tensor_scalar(dst, ps, bias, 0.0, op0=add, op1=amax)
                else:
                    nc.vector.tensor_scalar(dst, ps, bias, None, op0=add)

            def mlp_hidden(cur, w_sb, b_sb, pool, nt, layer, eng):
                nxt = pool.tile([128, nt], bf16, tag="h")
                for nb in range(nt // IBLK):
                    sl = slice(nb * IBLK, (nb + 1) * IBLK)
                    ps = ps_mlp.tile([128, IBLK], f32, tag="ps")
                    nc.tensor.matmul(
                        ps[:], w_sb[:, layer, :], cur[:, sl], start=True, stop=True
                    )
                    epilogue(nxt[:, sl], ps[:], b_sb[:, layer : layer + 1], layer, eng)
                return nxt

            hk, hq = xkT_sb, xqT_sb
            for layer in range(NLIN - 1):
                hk = mlp_hidden(hk, wk_sb, bk_sb, hkp, T // 2, layer, "act")
                hq = mlp_hidden(hq, wq_sb, bq_sb, hqp, NQ // 2, layer, "dve")

            # final K layer: write straight into block-diagonal pair tiles
            eng_flip = 0
            for nb in range(T // 2 // IBLK):
                sl = slice(nb * IBLK, (nb + 1) * IBLK)
                ps = ps_mlp.tile([128, IBLK], f32, tag="ps")
                nc.tensor.matmul(
                    ps[:], wk_sb[:, NLIN - 1, :], hk[:, sl], start=True, stop=True
                )
                psv = ps[:].rearrange("p (a e) -> p a e", e=64)
                pair = slice(8 * nb, 8 * nb + 8)
                bias = bk_sb[:, NLIN - 1 : NLIN]
                for half, csl in ((slice(0, 64), slice(0, 64)),
                                  (slice(64, 128), slice(64, 128))):
                    dst = kTblk[half, pair, csl]
                    src = psv[half, :, :]
                    if eng_flip % 2 == 0:
                        nc.scalar.activation(dst, src, Identity, bias=bias[half])
                    else:
                        nc.vector.tensor_scalar(dst, src, bias[half], None, op0=add)
                    eng_flip += 1

            # final Q layer: replicate Q^T onto both partition halves (bf16)
            # plus an fp32 copy of the top half for the base matmuls
            for nb in range(NQ // 2 // IBLK):
                sl = slice(nb * IBLK, (nb + 1) * IBLK)
                bias = bq_sb[:, NLIN - 1 : NLIN]
                for rep in range(2):
                    ps = ps_mlp.tile([128, IBLK], f32, tag="ps")
                    nc.tensor.matmul(
                        ps[:], wq_sb[:, 2 + rep, :], hq[:, sl], start=True, stop=True
                    )
                    osl = slice(rep * (NQ // 2) + nb * IBLK,
                                rep * (NQ // 2) + (nb + 1) * IBLK)
                    epilogue(qT2[:, osl], ps[:], bias, NLIN - 1,
                             "act" if rep else "dve")
                    epilogue(qTf[:, osl], ps[0:64, :], bias[0:64], NLIN - 1,
                             "dve" if rep else "act")

            # ---- delta pass: Knat tiles from packed hidden state, then
            # delta_t = Knat_t^T @ V_t into one PSUM bank [64, NPAIR*2]
            dps = ps_d.tile([64, NPAIR * 2], f32)
            kns = []
            for cc in range(16):
                psKt = ps_mlp.tile([128, 128], f32, tag="ps")
                hs = hk[:, 128 * cc : 128 * cc + 128]
                nc.tensor.matmul(
                    psKt[:], hs, wk_sb[:, 2, :],
                    start=True, stop=True, skip_group_check=True,
                )
                kn = knp.tile([128, 128], bf16, tag="kn")
                if cc % 2 == 0:
                    nc.scalar.copy(kn[:], psKt[:])
                else:
                    nc.vector.tensor_copy(kn[:], psKt[:])
                kns.append(kn)
            # all delta matmuls back-to-back into one PSUM bank
            for cc in range(16):
                kn = kns[cc]
                for h, (hp, vc) in enumerate(
                    ((slice(0, 64), slice(0, 2)), (slice(0, 64), slice(2, 4)),
                     (slice(64, 128), slice(0, 2)), (slice(64, 128), slice(2, 4)))
                ):
                    t = 2 * cc + (h // 2)
                    fc = slice(0, 64) if h % 2 == 0 else slice(64, 128)
                    nc.tensor.matmul(
                        dps[:, 2 * t : 2 * t + 2], kn[hp, fc], xkvT_sb[hp, cc, vc],
                        start=(cc == 0 and h == 0),
                        stop=(cc == 15 and h == 3), skip_group_check=True,
                    )

            # ---- prefix chain on DVE: PAll[c] = PAll[c-1] + delta_{c-1}
            for c in range(1, NPAIR + 1):
                nc.vector.tensor_add(
                    PAll[:, c, :], PAll[:, c - 1, :],
                    dps[:, 2 * (c - 1) : 2 * c],
                )

            # ---- main loop: per 512-block bank; base matmuls + boundary units
            for b in range(NBLK):
                bsl = slice(b * IBLK, (b + 1) * IBLK)
                ov = ps_o.tile([2, IBLK], f32)
                nc.vector.memset(ov[:], 0.0)
                units = []
                for q in range(4):
                    qc = 4 * b + q
                    for jt in range(F128[qc], J128[qc]):
                        units.append((q, qc, jt))
                # base matmuls (start=False over the memset bank)
                for q in range(4):
                    qc = 4 * b + q
                    osl = slice(q * QC, (q + 1) * QC)
                    nc.tensor.matmul(
                        ov[:, osl], PAll[:, F128[qc], :],
                        qTf[:, qc * QC : (qc + 1) * QC],
                        start=False, stop=False, skip_group_check=True,
                    )
                # boundary units: S-matmuls and AVs in runs of 3 so each
                # kind streams back-to-back on the PE
                pend = deque()

                def emit_av(q, jt, s_sb, last):
                    osl = slice(q * QC, (q + 1) * QC)
                    nc.tensor.matmul(
                        ov[:, osl], xkv_sb[:, jt, :], s_sb[:],
                        start=False, stop=last, skip_group_check=True,
                    )

                for i, (q, qc, jt) in enumerate(units):
                    qsl = slice(qc * QC, (qc + 1) * QC)
                    sp = ps_s.tile([CHUNK, QC], f32)
                    nc.tensor.matmul(
                        sp[:], kTblk[:, jt, :], qT2[:, qsl],
                        start=True, stop=True, skip_group_check=True,
                    )
                    s_sb = spool.tile([CHUNK, QC], bf16)
                    nc.vector.scalar_tensor_tensor(
                        s_sb[:], t1b_sb[:, qsl], xt2_sb[:, jt : jt + 1],
                        sp[:], op0=is_ge, op1=mult,
                    )
                    pend.append((q, jt, s_sb, i == len(units) - 1))
                    if i % 3 == 2:
                        while len(pend) > 3:
                            emit_av(*pend.popleft())
                while pend:
                    emit_av(*pend.popleft())
                nc.scalar.copy(out_sb[:, bsl], ov[:])

            nc.sync.dma_start(out[:], out_sb[:])

    nc.compile()
    return nc


def _stack_keys(a):
    """[T, ...] -> even/odd 64-chunk split stacked on a new leading axis."""
    v = a.reshape(NPAIR, 2, 64, *a.shape[1:])
    return v[:, 0], v[:, 1]  # each [NPAIR, 64, ...]


def kernel(x1, x2, x3, x4, Wq_w, Wq_b, Wk_w, Wk_b):
    from concourse.bass_utils import run_bass_kernel_spmd

    global LAST_RESULTS

    xs = [np.asarray(a, dtype=np.float32)[0, 0] for a in (x1, x2, x3, x4)]
    Wq_w = np.asarray(Wq_w, dtype=np.float32)
    Wq_b = np.asarray(Wq_b, dtype=np.float32)
    Wk_w = np.asarray(Wk_w, dtype=np.float32)
    Wk_b = np.asarray(Wk_b, dtype=np.float32)

    t1 = xs[0][:, -1]
    t2s = [x[:, -1] for x in xs]

    # ---- universal chunk classification (exact, quantified over all cores)
    F128 = [NPAIR] * NQC
    J128 = [0] * NQC
    for qc in range(NQC):
        for p in range(2):
            oc = 2 * qc + p
            lo, hi = t1[128 * oc], t1[128 * oc + 127]
            for m in range(M):
                nfull = int(np.searchsorted(t2s[m], lo, side="right"))
                nvis = int(np.searchsorted(t2s[m], hi, side="right"))
                F128[qc] = min(F128[qc], nfull // CHUNK)
                J128[qc] = max(J128[qc], -(-nvis // CHUNK))
        J128[qc] = max(J128[qc], F128[qc])

    nc = _build_program(F128, J128)

    # ---- host packing
    perm = np.empty((2, NQ), dtype=np.int64)
    for p in range(2):
        perm[p] = np.concatenate(
            [np.arange(128 * (2 * k + p), 128 * (2 * k + p) + 128) for k in range(16)]
        )

    def blockdiag(Wl):
        b = np.zeros((128, 128), np.float32)
        b[:64, :64] = Wl
        b[64:, 64:] = Wl
        return b

    # Q weights: layers 0,1 blockdiag; final as [[W,W],[0,0]] and [[0,0],[W,W]]
    wq_h = np.zeros((4, 128, 128), np.float32)
    for l in range(NLIN - 1):
        wq_h[l] = blockdiag(Wq_w[l])
    wq_h[2, :64, :64] = Wq_w[2]
    wq_h[2, :64, 64:] = Wq_w[2]
    wq_h[3, 64:, :64] = Wq_w[2]
    wq_h[3, 64:, 64:] = Wq_w[2]
    wq_h = np.ascontiguousarray(wq_h.transpose(1, 0, 2).reshape(128, 4 * 128))
    bq_h = np.tile(Wq_b.T, (2, 1))  # [128, 3]
    bq_h = np.ascontiguousarray(
        np.concatenate([bq_h, bq_h[:, 2:3]], axis=1)
    )  # [128, 4]

    x1T = np.ascontiguousarray(xs[0].T)

    in_maps = []
    for c in range(8):
        m, p = c // 2, c % 2
        xm = xs[m]
        # key-side stacking: even/odd 64-chunks
        ev, od = _stack_keys(xm)  # [NPAIR, 64, D] each
        xkT_h = np.concatenate(
            [
                ev.reshape(T // 2, D).T,   # [64, 2048]
                od.reshape(T // 2, D).T,
            ],
            axis=0,
        )  # [128, 2048]
        xkv_h = np.concatenate(
            [ev[:, :, 0:2], od[:, :, 0:2]], axis=1
        )  # [NPAIR, 128, 2]
        xkv_h = np.ascontiguousarray(
            xkv_h.transpose(1, 0, 2).reshape(128, NPAIR * 2)
        ).astype(ml_dtypes.bfloat16)
        v4 = xm[:, 0:2].reshape(16, 4, 64, 2)   # [chunk256, B, 64, 2]
        xkvT_h = np.concatenate(
            [np.concatenate([v4[:, 0], v4[:, 2]], axis=1)[:, :, None, :],
             np.concatenate([v4[:, 1], v4[:, 3]], axis=1)[:, :, None, :]],
            axis=2,
        )  # [16, 128, 2pairs, 2]
        xkvT_h = np.ascontiguousarray(
            xkvT_h.transpose(1, 0, 2, 3).reshape(128, 16 * 4)
        ).astype(ml_dtypes.bfloat16)
        xt2_h = np.concatenate(
            [ev[:, :, D - 1], od[:, :, D - 1]], axis=1
        ).T  # [128, NPAIR]

        wk_h = np.stack([blockdiag(Wk_w[m][l]) for l in range(NLIN)])
        wk_h = np.ascontiguousarray(wk_h.transpose(1, 0, 2).reshape(128, NLIN * 128))
        bk_h = np.ascontiguousarray(np.tile(Wk_b[m].T, (2, 1)))  # [128, 3]

        # query-side: parity packing then [first half | second half] stacking
        xq = x1T[:, perm[p]]  # [64, 2048]
        xqT_h = np.concatenate([xq[:, : NQ // 2], xq[:, NQ // 2 :]], axis=0)

        in_maps.append(
            {
                "xqT": np.ascontiguousarray(xqT_h).astype(ml_dtypes.bfloat16),
                "xkT": np.ascontiguousarray(xkT_h).astype(ml_dtypes.bfloat16),
                "xkv": xkv_h,
                "xkvT": xkvT_h,
                "xt2": np.ascontiguousarray(xt2_h),
                "t1p": np.ascontiguousarray(t1[perm[p]][None, :]),
                "wq": wq_h.astype(ml_dtypes.bfloat16),
                "bq": bq_h,
                "wk": wk_h.astype(ml_dtypes.bfloat16),
                "bk": bk_h,
            }
        )

    res = run_bass_kernel_spmd(nc, in_maps, core_ids=list(range(8)))
    LAST_RESULTS = res

    # ---- gather: sum over modalities, unpermute parity chunks, transpose
    acc = np.zeros((2, T), dtype=np.float32)
    for c in range(8):
        m, p = c // 2, c % 2
        acc[:, perm[p]] += res.results[c]["out"]
    out = np.ascontiguousarray(acc.T)[None]

    # ---- exact host correction for a nonzero final K bias in the base
    # region (the device prefix is bias-free; zero for reference inputs)
    if np.any(Wk_b[:, NLIN - 1]):
        def mlp(x, Ws, bs):
            h = x
            for i in range(Ws.shape[0]):
                h = h @ Ws[i] + bs[i]
                if i < Ws.shape[0] - 1:
                    h = np.maximum(h, 0.0)
            return h

        Q = mlp(xs[0], Wq_w, Wq_b)
        for m in range(M):
            qb = Q @ Wk_b[m][NLIN - 1]          # (T,)
            csV = np.concatenate(
                [np.zeros((1, 2), np.float32),
                 np.cumsum(xs[m][:, :2].reshape(NPAIR, CHUNK, 2).sum(1), axis=0)]
            )  # (NPAIR+1, 2)
            for p in range(2):
                for qc in range(NQC):
                    idx = perm[p][qc * QC : (qc + 1) * QC]
                    out[0, idx] += qb[idx, None] * csV[F128[qc]][None, :]
    return out


# revision 21
# speedup vs baseline: 2.1397x; 1.0037x over previous
"""Trainium2 Bass kernel for masked multi-modal causal dot-product attention.

Computation (reference):
  Q = mlp(x1, Wq)               # (4096, 64), 3 linear layers, relu between
  for m in 0..3:
    K_m = mlp(x_m, Wk[m])       # (4096, 64)
    mask_m[i,j] = t2_m[j] <= t1[i]   (timestamps sorted -> staircase mask)
    acc += ((Q @ K_m.T) * mask_m) @ x_m[:, :2]
  out = acc  # (1, 4096, 2)

Sharding: 8 cores = 4 modalities x 2 query-parity halves (queries interleaved
by 128-chunks for load balance). One SPMD program; per-core variation lives in
the input tensors.

Algorithm (prefix-sum restructure): since both t1 and t2 are sorted, the mask
is a monotone staircase. For each 128-query chunk, key tiles split into
fully-visible / boundary / invisible. The fully-visible mass uses
associativity:  sum_vis (Q.K_j) V_j = Q @ P  with  P = sum_j K_j^T V_j
a prefix sum over 128-key tiles of tiny [64,2] matrices. Only the ~2-4
boundary tiles per query chunk need explicit masked S tiles ([128,128]).

Device pipeline per core:
  - PE warm-up matmuls while DMAs stream (HAM un-throttle).
  - MLPs on stacked halves with block-diagonal weights (K=128 packing),
    f32r; final layers emit K^T pair tiles (kTblk) and Q^T (qT2) in bf16,
    plus an fp32 Q^T copy (qTf) for the base matmuls.
  - Delta pass: K-natural tiles reconstructed from the packed hidden state
    via two placed matmuls per pair tile (even/odd keys on partition
    halves), then delta_t = Knat_t^T @ V_t accumulated into one PSUM bank.
  - Prefix chain on DVE -> PAll[c] = sum_{t<c} delta_t  (fp32).
  - Per 512-query block: PSUM bank memset; 4 base matmuls (PAll[F]^T @ Q^T,
    f32r); boundary units: S tile (bf16) -> fused (t1>=t2)*S on DVE ->
    AV matmul; all accumulate into the same bank; copy out.

Final-layer K bias is folded exactly: boundary tiles use biased K (kTblk);
the base region correction (Q.b2)*prefix(sum V) is identically zero here
(reference biases are zero) but is applied on host if ever nonzero.
"""

import os
import sys
from collections import deque

import ml_dtypes
import numpy as np

sys.path.insert(0, "/opt/trn_rl_repo")

T = 4096
D = 64
M = 4
NLIN = 3
NQ = 2048          # packed queries per core
CHUNK = 128        # keys per pair tile (64 even + 64 odd)
NPAIR = T // CHUNK  # 32 pair tiles
IBLK = 512         # query block for MLPs / out banks
NBLK = NQ // IBLK  # 4 query blocks per core
QC = 128           # boundary query-chunk granularity
NQC = NQ // QC     # 16 query chunks per core

LAST_RESULTS = None


def _build_program(F128, J128):
    """F128[qc]: pair tiles < F128 are fully visible for every core's chunk
    qc; F128 <= jt < J128[qc] get the on-device mask (universal bounds)."""
    import concourse.bacc as bacc
    import concourse.mybir as mybir
    import concourse.tile as tile

    f32 = mybir.dt.float32
    f32r = mybir.dt.float32r
    bf16 = mybir.dt.bfloat16
    Relu = mybir.ActivationFunctionType.Relu
    Identity = mybir.ActivationFunctionType.Identity
    is_ge = mybir.AluOpType.is_ge
    add = mybir.AluOpType.add
    amax = mybir.AluOpType.max
    mult = mybir.AluOpType.mult

    nc = bacc.Bacc("TRN2", target_bir_lowering=False, debug=False, num_devices=8)

    xqT = nc.dram_tensor("xqT", [128, NQ // 2], bf16, kind="ExternalInput")
    xkT = nc.dram_tensor("xkT", [128, T // 2], bf16, kind="ExternalInput")
    xkv = nc.dram_tensor("xkv", [128, NPAIR * 2], bf16, kind="ExternalInput")
    xkvT = nc.dram_tensor("xkvT", [128, 16 * 4], bf16, kind="ExternalInput")
    xt2 = nc.dram_tensor("xt2", [128, NPAIR], f32, kind="ExternalInput")
    t1p = nc.dram_tensor("t1p", [1, NQ], f32, kind="ExternalInput")
    wq = nc.dram_tensor("wq", [128, 4 * 128], bf16, kind="ExternalInput")
    bq = nc.dram_tensor("bq", [128, 4], f32, kind="ExternalInput")
    wk = nc.dram_tensor("wk", [128, NLIN * 128], bf16, kind="ExternalInput")
    bk = nc.dram_tensor("bk", [128, NLIN], f32, kind="ExternalInput")
    out = nc.dram_tensor("out", [2, NQ], f32, kind="ExternalOutput")

    def rr(ap):
        return ap.bitcast(f32r)

    with tile.TileContext(nc) as tc:
        with (
            tc.tile_pool(name="const", bufs=1) as const,
            tc.tile_pool(name="hq", bufs=2) as hqp,
            tc.tile_pool(name="hk", bufs=2) as hkp,
            tc.tile_pool(name="knp", bufs=16) as knp,
            tc.tile_pool(name="spool", bufs=4) as spool,
            tc.tile_pool(name="mkp", bufs=4) as mkp,
            tc.tile_pool(name="ps_mlp", bufs=2, space="PSUM") as ps_mlp,
            tc.tile_pool(name="ps_d", bufs=1, space="PSUM") as ps_d,
            tc.tile_pool(name="ps_s", bufs=3, space="PSUM") as ps_s,
            tc.tile_pool(name="ps_o", bufs=1, space="PSUM") as ps_o,
        ):
            # ---- PE warm-up: dummy bf16 matmuls so HAM un-throttles the PE
            # clock (4/8 -> 8/8) while the input DMAs stream in.
            wu = const.tile([128, 512], bf16)
            nc.gpsimd.memset(wu[:], 1.0)
            for i in range(10):
                wps = ps_mlp.tile([128, 512], f32, tag="ps")
                nc.tensor.matmul(
                    wps[:], wu[:, 0:128], wu[:], start=True, stop=True,
                    skip_group_check=True,
                )

            # ---- inputs -> SBUF (ordered so the MLPs can start ASAP)
            wq_sb = const.tile([128, 4, 128], bf16)
            nc.sync.dma_start(wq_sb[:], wq[:].rearrange("p (l e) -> p l e", l=4))
            bq_sb = const.tile([128, 4], f32)
            nc.sync.dma_start(bq_sb[:], bq[:])
            wk_sb = const.tile([128, NLIN, 128], bf16)
            nc.sync.dma_start(wk_sb[:], wk[:].rearrange("p (l e) -> p l e", l=NLIN))
            bk_sb = const.tile([128, NLIN], f32)
            nc.sync.dma_start(bk_sb[:], bk[:])

            xqT_sb = const.tile([128, NQ // 2], bf16)
            xkT_sb = const.tile([128, T // 2], bf16)
            order = [("k", 0), ("k", 1), ("q", 0), ("k", 2), ("k", 3), ("q", 1)]
            for which, nb in order:
                sl = slice(nb * IBLK, (nb + 1) * IBLK)
                if which == "k":
                    nc.sync.dma_start(xkT_sb[:, sl], xkT[:, sl])
                else:
                    nc.sync.dma_start(xqT_sb[:, sl], xqT[:, sl])

            xkv_sb = const.tile([128, NPAIR, 2], bf16)
            nc.sync.dma_start(xkv_sb[:], xkv[:].rearrange("p (c f) -> p c f", f=2))
            xkvT_sb = const.tile([128, 16, 4], bf16)
            nc.sync.dma_start(xkvT_sb[:], xkvT[:].rearrange("p (c f) -> p c f", f=4))
            xt2_sb = const.tile([128, NPAIR], f32)
            nc.sync.dma_start(xt2_sb[:], xt2[:])
            t1b_sb = const.tile([CHUNK, NQ], f32)
            for nb in range(NBLK):
                sl = slice(nb * IBLK, (nb + 1) * IBLK)
                nc.sync.dma_start(t1b_sb[:, sl], t1p[:, sl].partition_broadcast(CHUNK))

            out_sb = const.tile([2, NQ], f32)

            # ---- blocked K^T target: pair tiles with block-diagonal layout
            kTblk = const.tile([128, NPAIR, CHUNK], bf16)
            zeros_sb = const.tile([128, NPAIR, 64], bf16)
            nc.vector.memset(zeros_sb[:], 0.0)
            nc.vector.tensor_copy(kTblk[0:64, :, 64:128], zeros_sb[0:64])
            nc.scalar.copy(kTblk[64:128, :, 0:64], zeros_sb[64:128])
            qT2 = const.tile([128, NQ], bf16)
            qTf = const.tile([64, NQ], f32r)
            PAll = const.tile([64, NPAIR + 1, 2], f32r)

            nc.vector.tensor_copy(PAll[:, 0, :], zeros_sb[0:64, 0, 0:2])

            # ---- stacked MLPs (block-diagonal weights, both halves at once)
            def epilogue(dst, ps, bias, layer, eng):
                if eng == "act":
                    func = Relu if layer < NLIN - 1 else Identity
                    nc.scalar.activation(dst, ps, func, bias=bias)
                elif layer < NLIN - 1:
                    nc.vector.tensor_scalar(dst, ps, bias, 0.0, op0=add, op1=amax)
                else:
                    nc.vector.tensor_scalar(dst, ps, bias, None, op0=add)

            def mlp_hidden(cur, w_sb, b_sb, pool, nt, layer, eng, keep_warm=False):
                nxt = pool.tile([128, nt], bf16, tag="h")
                for nb in range(nt // IBLK):
                    if keep_warm:
                        # independent matmul that runs while the chunk's DMA
                        # is in flight, keeping the HAM clock at 8/8
                        wps = ps_mlp.tile([128, 512], f32, tag="ps")
                        nc.tensor.matmul(
                            wps[:], wu[:, 0:128], wu[:], start=True, stop=True,
                            skip_group_check=True,
                        )
                    sl = slice(nb * IBLK, (nb + 1) * IBLK)
                    ps = ps_mlp.tile([128, IBLK], f32, tag="ps")
                    nc.tensor.matmul(
                        ps[:], w_sb[:, layer, :], cur[:, sl], start=True, stop=True
                    )
                    epilogue(nxt[:, sl], ps[:], b_sb[:, layer : layer + 1], layer, eng)
                return nxt

            hk, hq = xkT_sb, xqT_sb
            for layer in range(NLIN - 1):
                hk = mlp_hidden(hk, wk_sb, bk_sb, hkp, T // 2, layer, "act",
                                keep_warm=(layer == 0))
                hq = mlp_hidden(hq, wq_sb, bq_sb, hqp, NQ // 2, layer, "dve",
                                keep_warm=(layer == 0))

            # final K layer: write straight into block-diagonal pair tiles
            eng_flip = 0
            for nb in range(T // 2 // IBLK):
                sl = slice(nb * IBLK, (nb + 1) * IBLK)
                ps = ps_mlp.tile([128, IBLK], f32, tag="ps")
                nc.tensor.matmul(
                    ps[:], wk_sb[:, NLIN - 1, :], hk[:, sl], start=True, stop=True
                )
                psv = ps[:].rearrange("p (a e) -> p a e", e=64)
                pair = slice(8 * nb, 8 * nb + 8)
                bias = bk_sb[:, NLIN - 1 : NLIN]
                for half, csl in ((slice(0, 64), slice(0, 64)),
                                  (slice(64, 128), slice(64, 128))):
                    dst = kTblk[half, pair, csl]
                    src = psv[half, :, :]
                    if eng_flip % 2 == 0:
                        nc.scalar.activation(dst, src, Identity, bias=bias[half])
                    else:
                        nc.vector.tensor_scalar(dst, src, bias[half], None, op0=add)
                    eng_flip += 1

            # final Q layer: replicate Q^T onto both partition halves (bf16)
            # plus an fp32 copy of the top half for the base matmuls
            for nb in range(NQ // 2 // IBLK):
                sl = slice(nb * IBLK, (nb + 1) * IBLK)
                bias = bq_sb[:, NLIN - 1 : NLIN]
                for rep in range(2):
                    ps = ps_mlp.tile([128, IBLK], f32, tag="ps")
                    nc.tensor.matmul(
                        ps[:], wq_sb[:, 2 + rep, :], hq[:, sl], start=True, stop=True
                    )
                    osl = slice(rep * (NQ // 2) + nb * IBLK,
                                rep * (NQ // 2) + (nb + 1) * IBLK)
                    epilogue(qT2[:, osl], ps[:], bias, NLIN - 1,
                             "act" if rep else "dve")
                    epilogue(qTf[:, osl], ps[0:64, :], bias[0:64], NLIN - 1,
                             "dve" if rep else "act")

            # ---- delta pass: Knat tiles from packed hidden state, then
            # delta_t = Knat_t^T @ V_t into one PSUM bank [64, NPAIR*2]
            dpsA = ps_d.tile([64, NPAIR], f32, tag="da")
            dpsB = ps_d.tile([64, NPAIR], f32, tag="db")
            kns = []
            for cc in range(16):
                psKt = ps_mlp.tile([128, 128], f32, tag="ps")
                hs = hk[:, 128 * cc : 128 * cc + 128]
                nc.tensor.matmul(
                    psKt[:], hs, wk_sb[:, 2, :],
                    start=True, stop=True, skip_group_check=True,
                )
                kn = knp.tile([128, 128], bf16, tag="kn")
                if cc % 2 == 0:
                    nc.scalar.copy(kn[:], psKt[:])
                else:
                    nc.vector.tensor_copy(kn[:], psKt[:])
                kns.append(kn)
            # delta matmuls: even tiles -> bank A rows 0:64, odd tiles ->
            # bank B rows 64:128; adjacent pairs use distinct row groups and
            # banks so the PE runs them concurrently
            for cc in range(16):
                kn = kns[cc]
                for h in (0, 2, 1, 3):
                    hp = slice(0, 64) if h < 2 else slice(64, 128)
                    vc = slice(0, 2) if h % 2 == 0 else slice(2, 4)
                    fc = slice(0, 64) if h % 2 == 0 else slice(64, 128)
                    dst = dpsA if h < 2 else dpsB
                    t = 2 * cc + (0 if h < 2 else 1)
                    csl = slice(2 * (t // 2), 2 * (t // 2) + 2)
                    nc.tensor.matmul(
                        dst[:, csl], kn[hp, fc], xkvT_sb[hp, cc, vc],
                        start=(cc == 0 and h in (0, 2)),
                        stop=(cc == 15 and h in (1, 3)), skip_group_check=True,
                    )

            # ---- prefix chain on DVE: PAll[c] = PAll[c-1] + delta_{c-1}
            for c in range(1, NPAIR + 1):
                t = c - 1
                bank = dpsA if t % 2 == 0 else dpsB
                csl = slice(2 * (t // 2), 2 * (t // 2) + 2)
                nc.vector.tensor_add(
                    PAll[:, c, :], PAll[:, c - 1, :], bank[:, csl],
                )

            # ---- main loop: per 512-block bank; base matmuls + boundary units
            for b in range(NBLK):
                bsl = slice(b * IBLK, (b + 1) * IBLK)
                ov = ps_o.tile([2, IBLK], f32)
                nc.vector.memset(ov[:], 0.0)
                units = []
                for q in range(4):
                    qc = 4 * b + q
                    for jt in range(F128[qc], J128[qc]):
                        units.append((q, qc, jt))
                # base matmuls (start=False over the memset bank)
                for q in range(4):
                    qc = 4 * b + q
                    osl = slice(q * QC, (q + 1) * QC)
                    nc.tensor.matmul(
                        ov[:, osl], PAll[:, F128[qc], :],
                        qTf[:, qc * QC : (qc + 1) * QC],
                        start=False, stop=False, skip_group_check=True,
                    )
                # boundary units: S-matmuls and AVs in runs of 3 so each
                # kind streams back-to-back on the PE
                pend = deque()

                def emit_av(q, jt, s_sb, last):
                    osl = slice(q * QC, (q + 1) * QC)
                    nc.tensor.matmul(
                        ov[:, osl], xkv_sb[:, jt, :], s_sb[:],
                        start=False, stop=last, skip_group_check=True,
                    )

                for i, (q, qc, jt) in enumerate(units):
                    qsl = slice(qc * QC, (qc + 1) * QC)
                    sp = ps_s.tile([CHUNK, QC], f32)
                    nc.tensor.matmul(
                        sp[:], kTblk[:, jt, :], qT2[:, qsl],
                        start=True, stop=True, skip_group_check=True,
                    )
                    s_sb = spool.tile([CHUNK, QC], bf16)
                    nc.vector.scalar_tensor_tensor(
                        s_sb[:], t1b_sb[:, qsl], xt2_sb[:, jt : jt + 1],
                        sp[:], op0=is_ge, op1=mult,
                    )
                    pend.append((q, jt, s_sb, i == len(units) - 1))
                    if i % 3 == 2:
                        while len(pend) > 3:
                            emit_av(*pend.popleft())
                while pend:
                    emit_av(*pend.popleft())
                nc.scalar.copy(out_sb[:, bsl], ov[:])

            nc.sync.dma_start(out[:], out_sb[:])

    nc.compile()
    return nc


def _stack_keys(a):
    """[T, ...] -> even/odd 64-chunk split stacked on a new leading axis."""
    v = a.reshape(NPAIR, 2, 64, *a.shape[1:])
    return v[:, 0], v[:, 1]  # each [NPAIR, 64, ...]


def kernel(x1, x2, x3, x4, Wq_w, Wq_b, Wk_w, Wk_b):
    from concourse.bass_utils import run_bass_kernel_spmd

    global LAST_RESULTS

    xs = [np.asarray(a, dtype=np.float32)[0, 0] for a in (x1, x2, x3, x4)]
    Wq_w = np.asarray(Wq_w, dtype=np.float32)
    Wq_b = np.asarray(Wq_b, dtype=np.float32)
    Wk_w = np.asarray(Wk_w, dtype=np.float32)
    Wk_b = np.asarray(Wk_b, dtype=np.float32)

    t1 = xs[0][:, -1]
    t2s = [x[:, -1] for x in xs]

    # ---- universal chunk classification (exact, quantified over all cores)
    F128 = [NPAIR] * NQC
    J128 = [0] * NQC
    for qc in range(NQC):
        for p in range(2):
            oc = 2 * qc + p
            lo, hi = t1[128 * oc], t1[128 * oc + 127]
            for m in range(M):
                nfull = int(np.searchsorted(t2s[m], lo, side="right"))
                nvis = int(np.searchsorted(t2s[m], hi, side="right"))
                F128[qc] = min(F128[qc], nfull // CHUNK)
                J128[qc] = max(J128[qc], -(-nvis // CHUNK))
        J128[qc] = max(J128[qc], F128[qc])

    nc = _build_program(F128, J128)

    # ---- host packing
    perm = np.empty((2, NQ), dtype=np.int64)
    for p in range(2):
        perm[p] = np.concatenate(
            [np.arange(128 * (2 * k + p), 128 * (2 * k + p) + 128) for k in range(16)]
        )

    def blockdiag(Wl):
        b = np.zeros((128, 128), np.float32)
        b[:64, :64] = Wl
        b[64:, 64:] = Wl
        return b

    # Q weights: layers 0,1 blockdiag; final as [[W,W],[0,0]] and [[0,0],[W,W]]
    wq_h = np.zeros((4, 128, 128), np.float32)
    for l in range(NLIN - 1):
        wq_h[l] = blockdiag(Wq_w[l])
    wq_h[2, :64, :64] = Wq_w[2]
    wq_h[2, :64, 64:] = Wq_w[2]
    wq_h[3, 64:, :64] = Wq_w[2]
    wq_h[3, 64:, 64:] = Wq_w[2]
    wq_h = np.ascontiguousarray(wq_h.transpose(1, 0, 2).reshape(128, 4 * 128))
    bq_h = np.tile(Wq_b.T, (2, 1))  # [128, 3]
    bq_h = np.ascontiguousarray(
        np.concatenate([bq_h, bq_h[:, 2:3]], axis=1)
    )  # [128, 4]

    x1T = np.ascontiguousarray(xs[0].T)

    in_maps = []
    for c in range(8):
        m, p = c // 2, c % 2
        xm = xs[m]
        # key-side stacking: even/odd 64-chunks
        ev, od = _stack_keys(xm)  # [NPAIR, 64, D] each
        xkT_h = np.concatenate(
            [
                ev.reshape(T // 2, D).T,   # [64, 2048]
                od.reshape(T // 2, D).T,
            ],
            axis=0,
        )  # [128, 2048]
        xkv_h = np.concatenate(
            [ev[:, :, 0:2], od[:, :, 0:2]], axis=1
        )  # [NPAIR, 128, 2]
        xkv_h = np.ascontiguousarray(
            xkv_h.transpose(1, 0, 2).reshape(128, NPAIR * 2)
        ).astype(ml_dtypes.bfloat16)
        v4 = xm[:, 0:2].reshape(16, 4, 64, 2)   # [chunk256, B, 64, 2]
        xkvT_h = np.concatenate(
            [np.concatenate([v4[:, 0], v4[:, 2]], axis=1)[:, :, None, :],
             np.concatenate([v4[:, 1], v4[:, 3]], axis=1)[:, :, None, :]],
            axis=2,
        )  # [16, 128, 2pairs, 2]
        xkvT_h = np.ascontiguousarray(
            xkvT_h.transpose(1, 0, 2, 3).reshape(128, 16 * 4)
        ).astype(ml_dtypes.bfloat16)
        xt2_h = np.concatenate(
            [ev[:, :, D - 1], od[:, :, D - 1]], axis=1
        ).T  # [128, NPAIR]

        wk_h = np.stack([blockdiag(Wk_w[m][l]) for l in range(NLIN)])
        wk_h = np.ascontiguousarray(wk_h.transpose(1, 0, 2).reshape(128, NLIN * 128))
        bk_h = np.ascontiguousarray(np.tile(Wk_b[m].T, (2, 1)))  # [128, 3]

        # query-side: parity packing then [first half | second half] stacking
        xq = x1T[:, perm[p]]  # [64, 2048]
        xqT_h = np.concatenate([xq[:, : NQ // 2], xq[:, NQ // 2 :]], axis=0)

        in_maps.append(
            {
                "xqT": np.ascontiguousarray(xqT_h).astype(ml_dtypes.bfloat16),
                "xkT": np.ascontiguousarray(xkT_h).astype(ml_dtypes.bfloat16),
                "xkv": xkv_h,
                "xkvT": xkvT_h,
                "xt2": np.ascontiguousarray(xt2_h),
                "t1p": np.ascontiguousarray(t1[perm[p]][None, :]),
                "wq": wq_h.astype(ml_dtypes.bfloat16),
                "bq": bq_h,
                "wk": wk_h.astype(ml_dtypes.bfloat16),
                "bk": bk_h,
            }
        )

    res = run_bass_kernel_spmd(nc, in_maps, core_ids=list(range(8)))
    LAST_RESULTS = res

    # ---- gather: sum over modalities, unpermute parity chunks, transpose
    acc = np.zeros((2, T), dtype=np.float32)
    for c in range(8):
        m, p = c // 2, c % 2
        acc[:, perm[p]] += res.results[c]["out"]
    out = np.ascontiguousarray(acc.T)[None]

    # ---- exact host correction for a nonzero final K bias in the base
    # region (the device prefix is bias-free; zero for reference inputs)
    if np.any(Wk_b[:, NLIN - 1]):
        def mlp(x, Ws, bs):
            h = x
            for i in range(Ws.shape[0]):
                h = h @ Ws[i] + bs[i]
                if i < Ws.shape[0] - 1:
                    h = np.maximum(h, 0.0)
            return h

        Q = mlp(xs[0], Wq_w, Wq_b)
        for m in range(M):
            qb = Q @ Wk_b[m][NLIN - 1]          # (T,)
            csV = np.concatenate(
                [np.zeros((1, 2), np.float32),
                 np.cumsum(xs[m][:, :2].reshape(NPAIR, CHUNK, 2).sum(1), axis=0)]
            )  # (NPAIR+1, 2)
            for p in range(2):
                for qc in range(NQC):
                    idx = perm[p][qc * QC : (qc + 1) * QC]
                    out[0, idx] += qb[idx, None] * csV[F128[qc]][None, :]
    return out
